# revision 9
# baseline (speedup 1.0000x reference)
"""Trainium2 Bass kernel for nn_AttentionLayer_83545703842160.

Single-head attention over spatial tokens, per batch element:
  t = x[b].reshape(C, H*W)              # tokens in columns, N=4096, C=64
  q,k,v = W{q,k,v} t + b{q,k,v}
  out   = softmax(q.T k / sqrt(C)) @ v.T   -> [C, H, W]

Sharding: data-parallel over batch B=8 across the 8 NeuronCores (one
batch element per core).  Each core holds the full (tiny) QKV weights.

Algebra (softmax row-shift invariance shrinks the PE work):
  q_n.k_m = t_n'(Wq'Wk) t_m + (Wq'bk).t_n + (Wk'bq).t_m + bq.bk
The n-only terms cancel in softmax, so with A = Wq'Wk, w = Wk'bq and
homogeneous coords x~ = [t; 1]:
  softmax_m(q.k/8) = softmax_m(([t_n;1] . [A t_m; w.t_m]) / 8)
One projection stream kt_ext = [A t; w.t] replaces BOTH q and k
projections; the query side streams bf16(x~) directly (no PE work).
The v bias rides the ones row as usual; the denominator comes from a
ones column in the extended v.

Per-core kernel layout/schedule (4 superblock-PAIRS of 1024 queries):
  - xt [65, N] f32r: x[b] + ones row.  kt_ext [65, N] bf16 via 8 chunk
    matmuls against a single stationary Aext [65,65]; xq_ext [65, N]
    bf16 is a plain DVE cast of xt.
  - MM1: per key m-tile, ONE matmul [65 -> 128, 1024] covering two
    superblocks (halves MM1 instruction + weight-load count, doubles
    the uninterrupted moving stream).  sc [128, 1024] psum (2 banks,
    ping-pong).  ACT: exp(0.125 sc) -> pt [128, 1024] bf16.
  - stage-2 lags stage-1 by L=3 groups: acc[66, 1024] += v_ext[m].T @
    pt (two 512-wide matmuls sharing the same stationary v tile);
    row 64 accumulates the softmax denominator.  The post-loop drain is
    only L groups + the last normalize (~2us, vs 7us when stage-2
    lagged a full superblock).
  - tail per superblock: recip(denom) -> gpsimd partition_broadcast ->
    DVE multiply -> DMA out.
  PSUM: scores 2x2 banks + acc pair [66,1024] 2x1 + proj ring 2x1 = 8.
  Cross-pair acc reuse is given 2 extra lag groups so the normalize
  drains before the next pair's first accumulate (PE is in-order).

HW notes (probe-measured): the PE matmul stream, not ACT element
throughput, limits this kernel on HW; tile_position row-pairing and
tensor_scalar-with-AP lower incorrectly on this runtime, so v3 sticks
to baseline-proven constructs and wins via instruction/LDW count and
the removed q-projection / drain tail.
"""

import os
import numpy as np
from contextlib import ExitStack

import concourse.bacc as bacc
import concourse.bass as bass
import concourse.mybir as mybir
import concourse.tile as tile
from concourse.bass import MemorySpace
from concourse.bass_utils import run_bass_kernel_spmd

C = 64          # channels
N = 4096        # tokens (64*64 spatial)
B = 8           # batch == number of cores
S = 512         # query superblock
P2 = 2 * S      # superblock pair width
MT = 128        # keys per m-tile
NMT = N // MT   # 32 m-tiles == groups per pair
NPAIR = N // P2  # 4 superblock pairs
NG = NPAIR * NMT  # 128 groups total
L = 3           # stage-2 lag in groups
XL = 2          # extra lag for the first groups after a pair boundary
KC = C + 1      # contraction depth (homogeneous row)
WCOLS = KC + 1 + C + 2   # packed weights: [Aext (65+1 pad) | wv_ext (66)]
FP32 = mybir.dt.float32
F32R = mybir.dt.float32r
BF16 = mybir.dt.bfloat16
EXP = mybir.ActivationFunctionType.Exp

_PROBE = os.environ.get("KPROBE", "")  # timing probes: "act" | "pe" | ""


def _build_kernel(tc, ctx, x_d, w_d, y_d, reps=1):
    if reps > 1:
        # timing harness: repeat the whole body in a HW loop so kernel time
        # dominates dispatch overhead in wallclock measurements
        engines = (mybir.EngineType.PE, mybir.EngineType.Activation,
                   mybir.EngineType.DVE, mybir.EngineType.Pool,
                   mybir.EngineType.SP)
        with tc.For_i(0, reps, 1, hint_engines=engines):
            _build_body(tc, ctx, x_d, w_d, y_d)
    else:
        _build_body(tc, ctx, x_d, w_d, y_d)


def _build_body(tc, ctx, x_d, w_d, y_d):
    nc = tc.nc

    sb = ctx.enter_context(tc.tile_pool(name="sb", bufs=1))
    pt_pool = ctx.enter_context(tc.tile_pool(name="pt", bufs=8))
    osb_pool = ctx.enter_context(tc.tile_pool(name="osb", bufs=2))
    nrm_pool = ctx.enter_context(tc.tile_pool(name="nrm", bufs=2))
    sc_psum = ctx.enter_context(
        tc.tile_pool(name="scp", bufs=2, space=MemorySpace.PSUM))
    ac_psum = ctx.enter_context(
        tc.tile_pool(name="acp", bufs=1, space=MemorySpace.PSUM))
    pj_psum = ctx.enter_context(
        tc.tile_pool(name="pjp", bufs=2, space=MemorySpace.PSUM))

    xt = sb.tile([KC, N], F32R)            # x + ones row
    w_sb = sb.tile([KC, WCOLS], F32R)
    kt = sb.tile([KC, N], BF16)            # [A t; w.t]
    xq = sb.tile([KC, N], BF16)            # bf16([t; 1])
    v_sb = sb.tile([MT, NMT, C + 2], BF16)

    aext = w_sb[:, 0:KC]                   # [65, 65] stationary
    wv = w_sb[:, KC + 1:WCOLS]             # [65, 66]

    # DMA: chunk 0/3/4 + w on sync's queue; the rest stream on gpsimd's.
    nc.sync.dma_start(xt[:, 0:S], x_d[:, 0:S])
    nc.sync.dma_start(w_sb[:], w_d)
    for j, eng in ((1, nc.gpsimd), (2, nc.gpsimd), (3, nc.sync),
                   (4, nc.sync), (5, nc.gpsimd), (6, nc.gpsimd),
                   (7, nc.gpsimd)):
        eng.dma_start(xt[:, j * S:(j + 1) * S], x_d[:, j * S:(j + 1) * S])

    def emit_kt(j):
        p = pj_psum.tile([KC, S], FP32, tag="pj", name="p")
        nc.tensor.matmul(p[:], aext, xt[:, j * S:(j + 1) * S],
                         start=True, stop=True)
        nc.vector.tensor_copy(kt[:, j * S:(j + 1) * S], p[:])

    def emit_xq(j):  # pure DVE: bf16 cast of an xt chunk
        nc.vector.tensor_copy(xq[:, j * S:(j + 1) * S],
                              xt[:, j * S:(j + 1) * S])

    def emit_v(m):
        p = pj_psum.tile([MT, C + 2], FP32, tag="pj", name="p")
        nc.tensor.matmul(p[:], xt[:, m * MT:(m + 1) * MT], wv,
                         start=True, stop=True)
        nc.vector.tensor_copy(v_sb[:, m, :], p[:])

    def emit_tail(acc, s, half):
        # normalize: y[:, block] = acc[0:64, half] / acc[64, half]
        a = acc[:, half * S:(half + 1) * S]
        rs = nrm_pool.tile([1, S], FP32, tag="rs", name="rs")
        nc.vector.tensor_copy(rs[:], a[C:C + 1, :])
        rr = nrm_pool.tile([1, S], FP32, tag="rr", name="rr")
        nc.vector.reciprocal(rr[:], rs[:])
        bc = nrm_pool.tile([C, S], FP32, tag="bc", name="bc")
        nc.gpsimd.partition_broadcast(bc[:], rr[:], channels=C)
        ob = osb_pool.tile([C, S], FP32, tag="ob", name="ob")
        nc.vector.tensor_mul(ob[:], a[0:C, :], bc[:])
        nc.sync.dma_start(y_d[:, s * S:(s + 1) * S], ob[:])

    # head: first kt chunk + first xq pair-block
    emit_kt(0)
    emit_xq(0)
    emit_xq(1)

    SQ = S // 2 if _PROBE == "pe" else S
    pts = {}
    accs = {}

    def stage2(Gp):
        pp, mp = divmod(Gp, NMT)
        if mp == 0:
            accs[pp] = ac_psum.tile([C + 2, P2], FP32, tag="acc", name="acc")
        acc = accs[pp]
        ptp = pts.pop(Gp)
        for half in range(2):
            nc.tensor.matmul(acc[:, half * S:half * S + SQ], v_sb[:, mp, :],
                             ptp[:, half * S:half * S + SQ],
                             start=(mp == 0), stop=(mp == NMT - 1))
        if mp == NMT - 1:
            acc = accs.pop(pp)
            emit_tail(acc, 2 * pp, 0)
            emit_tail(acc, 2 * pp + 1, 1)

    # stage-2 emission schedule: group Gp runs at G = Gp + L, plus XL more
    # across pair boundaries so the previous pair's normalize reads drain
    # before this pair's acc reuses the banks (PE executes in order).
    s2_at = {}
    for Gp in range(NG):
        _, mp = divmod(Gp, NMT)
        s2_at.setdefault(Gp + L + (XL if mp < 2 else 0), []).append(Gp)

    for G in range(NG):
        p, m = divmod(G, NMT)
        sc = sc_psum.tile([MT, P2], FP32, tag="sc", name="sc")
        # two 512-wide matmuls (psum-bank limit) sharing one stationary
        for half in range(2):
            nc.tensor.matmul(sc[:, half * S:half * S + SQ],
                             kt[:, m * MT:(m + 1) * MT],
                             xq[:, p * P2 + half * S:p * P2 + half * S + SQ],
                             start=True, stop=True)
        for Gp in s2_at.get(G, ()):
            stage2(Gp)
        pt = pt_pool.tile([MT, P2], BF16, tag="pt", name="pt")
        if _PROBE == "act":
            nc.scalar.activation(pt[:, 0:S], sc[:, 0:S], EXP, scale=0.125)
        else:
            nc.scalar.activation(pt[:], sc[:], EXP, scale=0.125)
        pts[G] = pt
        # projection schedule (pair 0 has slack-heavy deadlines)
        if 1 <= G <= 7:
            emit_kt(G)
        if 1 <= G <= 16:
            emit_v(2 * (G - 1))
            emit_v(2 * (G - 1) + 1)
        if G in (8, 40, 72):
            blk = {8: 2, 40: 4, 72: 6}[G]
            emit_xq(blk)
            emit_xq(blk + 1)
    for G in range(NG, NG + L + XL):
        for Gp in s2_at.get(G, ()):
            stage2(Gp)


_NC_CACHE = {}


def _get_nc(reps=1):
    if reps not in _NC_CACHE:
        nc = bacc.Bacc("TRN2", target_bir_lowering=False, debug=False,
                       enable_asserts=False)
        x_d = nc.dram_tensor("x", [KC, N], F32R, kind="ExternalInput").ap()
        w_d = nc.dram_tensor("w", [KC, WCOLS], F32R,
                             kind="ExternalInput").ap()
        y_d = nc.dram_tensor("y", [C, N], FP32, kind="ExternalOutput").ap()
        with tile.TileContext(nc) as tc:
            with ExitStack() as ctx:
                _build_kernel(tc, ctx, x_d, w_d, y_d, reps=reps)
        nc.compile()
        _NC_CACHE[reps] = nc
    return _NC_CACHE[reps]


def _host_weights(Wq, bq, Wk, bk, Wv, bv):
    Wq = np.asarray(Wq, np.float32)
    bq = np.asarray(bq, np.float32)
    Wk = np.asarray(Wk, np.float32)
    Wv = np.asarray(Wv, np.float32)
    bv = np.asarray(bv, np.float32)
    A = Wq.T @ Wk                       # [64, 64]
    wvec = Wk.T @ bq                    # [64]
    w = np.zeros((KC, WCOLS), np.float32)
    # Aext: out[p,n] = sum_j Aext[j,p] xt[j,n]; rows j<64 from A/w, ones row 0
    w[0:C, 0:C] = A.T                   # kt rows 0..63 = A t
    w[0:C, C] = wvec                    # kt row 64 = w.t
    # wv_ext: v = Wv t + bv (bias via ones row), col 64 = denominator ones
    w[0:C, KC + 1:KC + 1 + C] = Wv.T
    w[C, KC + 1:KC + 1 + C] = bv
    w[C, KC + 1 + C] = 1.0
    return w


def _host_x(x_b):
    return np.ascontiguousarray(
        np.concatenate([x_b.reshape(C, N), np.ones((1, N), np.float32)],
                       axis=0))


def _run(inputs, reps=1, **spmd_kwargs):
    x = np.ascontiguousarray(np.asarray(inputs["x"], np.float32))
    w = _host_weights(
        inputs["Wq"], inputs["bq"], inputs["Wk"], inputs["bk"],
        inputs["Wv"], inputs["bv"])
    nc = _get_nc(reps)
    in_maps = [{"x": _host_x(x[b]), "w": w} for b in range(B)]
    res = run_bass_kernel_spmd(nc, in_maps, core_ids=list(range(B)),
                               **spmd_kwargs)
    out = np.stack([res.results[b]["y"].reshape(C, 64, 64)
                    for b in range(B)], axis=0)
    return out, res


def kernel(**inputs):
    out, _ = _run(inputs)
    return out
